# revision 21
# baseline (speedup 1.0000x reference)
"""Multi-head attention kernel for Trainium2 (Bass/Tile), 8-core SPMD.

Problem: x[2, 2048, 1024], 16 heads x 64 dims, boolean key mask (all ones
per spec), W_qkv[1024, 3072], W_out[1024, 1024]. Reference is fp32.

Sharding: core c -> (batch b = c // 4, head-group g = c % 4 of 4 heads).
Each core computes attention for its 4 heads of its batch and a partial
output projection [2048, 1024]; the host sums the 4 head-group partials
per batch (the tensor-parallel reduce, done at unshard time) and adds
b_out plus the V-bias correction (attention rows sum to 1, so the V bias
contributes exactly b_v @ W_out to every output row).

All matmul operands are fp16 (10-bit mantissa, same as TF32; every tensor
here is O(10) so range is fine); PSUM accumulation is fp32. Softmax runs
without max-subtraction (scores are O(3)), with the key mask folded in as
a per-partition additive bias on the exp.

Per-core dataflow:
  xt      [1024, 2048] fp16  x[b]^T                    (host-prepped)
  QT/KT   [128, 2, 2048]     pair block p holds heads (2p, 2p+1) stacked
                             64+64 on partitions, free = seq
  V_nat   [128, 16, 4, 65]   natural-layout V; per head 64 cols + 1 ones
                             col (denominator rides the AV matmul)
  scoresT [kpos, qpos] psum  row-group-packed K=64 matmul pairs
  exp     ACT psum->sbuf fp16, bias = mask bias (0 / -1e30) per kpos
  AV      lhsT = V_aug [128, 65] -> psum [65, 512]: rows 0-63 = outT^T
          unnormalized, row 64 = sum(exp)
  norm    1-op approx reciprocal (SBUF-sourced), K=1 ones-matmul
          broadcast across 64 partitions, DVE multiply
  outproj y[qpos 128, 1024] = outT^T @ W_out_local over 2 pair blocks

Phase order keeps the PE queue warm and lets exp start early:
K proj (all strips) -> V -> Q strip 0 -> [attention s; Q proj s+1] ...
"""

import sys

sys.path.insert(0, "/opt/trn_rl_repo")

import numpy as np

B, N, D = 2, 2048, 1024
HEADS, DH = 16, 64
SCALE = DH ** -0.5
NCORES = 8
GROUPS = 4                      # head groups (tensor parallel)
DLOC = (HEADS // GROUPS) * DH   # 256 local inner dims per core

_CACHE = {}


def build_model(debug_taps=False, with_bias=False):
    """Build (once) the single-core Bass/Tile program shared by all 8 cores.

    with_bias adds the Q/K bias matmuls (b_qkv is all-zero per the problem
    spec, so the default model omits them; kernel() picks the bias variant
    if it ever sees a nonzero b_qkv)."""
    key = ("nc", debug_taps, with_bias)
    if key in _CACHE:
        return _CACHE[key]

    from concourse import bacc, mybir, tile

    f32 = mybir.dt.float32
    f16 = mybir.dt.float16
    AF = mybir.ActivationFunctionType

    nc = bacc.Bacc("TRN2", target_bir_lowering=False, debug=False)

    xt_d = nc.dram_tensor("xt", [D, N], f16, kind="ExternalInput").ap()
    wqkv_d = nc.dram_tensor("wqkv", [D, 3 * DLOC], f16, kind="ExternalInput").ap()
    brow_d = nc.dram_tensor("brow", [1, 3 * DLOC], f16, kind="ExternalInput").ap()
    wout_d = nc.dram_tensor("wout", [DLOC, D], f16, kind="ExternalInput").ap()
    mb_d = nc.dram_tensor("mb", [N, 1], f32, kind="ExternalInput").ap()
    ones_d = nc.dram_tensor("ones_in", [128, 128], f16, kind="ExternalInput").ap()
    y_d = nc.dram_tensor("y", [N, D], f32, kind="ExternalOutput").ap()
    if debug_taps:
        dbg_dn = nc.dram_tensor("dbg_dn", [16, 512], f32, kind="ExternalOutput").ap()
        dbg_rc = nc.dram_tensor("dbg_rc", [16, 512], f32, kind="ExternalOutput").ap()

    DC = D // 128        # 8 contraction chunks
    QC = 4               # 512-wide query strips
    KC = N // 128        # 16 key chunks
    NPC = N // 128       # 16 seq chunks for V natural

    with tile.TileContext(nc) as tc:
        with (
            tc.tile_pool(name="resident", bufs=1) as res,
            tc.tile_pool(name="exp", bufs=6) as exp_pool,
            tc.tile_pool(name="ysb", bufs=2) as y_pool,
            tc.tile_pool(name="small", bufs=4) as small_pool,
            tc.tile_pool(name="ps", bufs=4, space="PSUM") as ps,
            tc.tile_pool(name="spair", bufs=2, space="PSUM") as ps_s,
        ):
            xt = res.tile([128, DC, N], f16)
            wqkv = res.tile([128, DC, 3 * DLOC], f16)
            brow = res.tile([1, 3 * DLOC], f16)
            wout = res.tile([128, 2, D], f16)
            mb = res.tile([128, KC], f32)
            ones = res.tile([1, 512], f16) if with_bias else None
            qt = res.tile([128, 2, N], f16)
            kt = res.tile([128, 2, N], f16)
            vn = res.tile([128, NPC, 4, 65], f16)
            outt = res.tile([128, 2, N], f16)

            if with_bias:
                nc.sync.dma_start(
                    ones[:],
                    ones_d.rearrange("a b -> (a b)")[0:512].unsqueeze(0),
                )
            nc.sync.dma_start(
                vn[:, :, :, 64:65],
                ones_d[:, 0:64].rearrange("p (j h) -> p j h", h=4).unsqueeze(-1),
            )

            # ---- input DMAs (K cols and first xt strip first)
            wqkv_src = wqkv_d.rearrange("(c p) w -> p c w", p=128)
            nc.sync.dma_start(wqkv[:, :, DLOC:2 * DLOC], wqkv_src[:, :, DLOC:2 * DLOC])
            xt_src = xt_d.rearrange("(c p) n -> p c n", p=128)
            for s in range(QC):
                nc.sync.dma_start(
                    xt[:, :, s * 512:(s + 1) * 512], xt_src[:, :, s * 512:(s + 1) * 512]
                )
            nc.sync.dma_start(wqkv[:, :, 2 * DLOC:3 * DLOC], wqkv_src[:, :, 2 * DLOC:3 * DLOC])
            nc.sync.dma_start(wqkv[:, :, 0:DLOC], wqkv_src[:, :, 0:DLOC])
            nc.sync.dma_start(brow[:], brow_d[:])
            nc.sync.dma_start(mb[:], mb_d.rearrange("(k p) one -> p (k one)", p=128))
            nc.sync.dma_start(wout[:], wout_d.rearrange("(c p) dd -> p c dd", p=128))

            def project_qk(tgt, dst, s):
                """One strip of the Q^T / K^T projection (both pair blocks)."""
                for p in range(2):
                    col0 = tgt * DLOC + p * 128
                    psum = ps.tile([128, 512], f32, tag="ps", name="qk_ps")
                    for c in range(DC):
                        nc.tensor.matmul(
                            psum[:],
                            wqkv[:, c, col0:col0 + 128],
                            xt[:, c, s * 512:(s + 1) * 512],
                            start=(c == 0),
                            stop=(not with_bias and c == DC - 1),
                        )
                    if with_bias:
                        nc.tensor.matmul(   # + per-partition bias via bias-row lhsT
                            psum[:],
                            brow[0:1, col0:col0 + 128],
                            ones[0:1, 0:512],
                            start=False,
                            stop=True,
                        )
                    nc.vector.tensor_copy(dst[:, p, s * 512:(s + 1) * 512], psum[:])

            # K first (scores need every K chunk), then Q strip 0.
            # V is woven into the first attention group as background tasks.
            for s in range(QC):
                project_qk(1, kt, s)
            project_qk(0, qt, 0)

            def vproj_task(j):
                psum = ps.tile([128, 256], f32, tag="ps", name="v_ps")
                for c in range(DC):
                    nc.tensor.matmul(
                        psum[:],
                        xt[:, c, j * 128:(j + 1) * 128],
                        wqkv[:, c, 2 * DLOC:3 * DLOC],
                        start=(c == 0),
                        stop=(c == DC - 1),
                    )
                nc.vector.tensor_copy(
                    vn[:, j, :, 0:64],
                    psum[:].rearrange("a (h x) -> a h x", h=4),
                )

            # ---- background PE task generators (interleaved into k-loops) ----
            def outproj_tasks(s):
                """8 tasks: output projection of strip s as (jj, nb) 2-MM groups."""
                state = {}
                tasks = []
                for jj in range(4):
                    for nb in range(2):
                        def t(jj=jj, nb=nb):
                            q0 = s * 512 + jj * 128
                            if nb == 0:
                                state[jj] = y_pool.tile([128, D], f32, tag="ysb", name="ysb")
                            ysb = state[jj]
                            yps = ps.tile([128, 512], f32, tag="ps", name="yps")
                            for p in range(2):
                                nc.tensor.matmul(
                                    yps[:],
                                    outt[:, p, q0:q0 + 128],
                                    wout[:, p, nb * 512:(nb + 1) * 512],
                                    start=(p == 0),
                                    stop=(p == 1),
                                )
                            nc.vector.tensor_copy(ysb[:, nb * 512:(nb + 1) * 512], yps[:])
                            if nb == 1:
                                nc.sync.dma_start(y_d[q0:q0 + 128, :], ysb[:])
                        tasks.append(t)
                return tasks

            def qproj_tasks(s):
                """6 tasks of <=3 MMs each: Q^T projection of strip s."""
                state = {}
                tasks = []
                for p in range(2):
                    for ci, chunk in enumerate(((0, 1, 2), (3, 4, 5), (6, 7, -1))):
                        def t(p=p, ci=ci, chunk=chunk):
                            col0 = p * 128
                            if ci == 0:
                                state[p] = ps.tile([128, 512], f32, tag="ps", name="qk_ps")
                            psum = state[p]
                            for c in chunk:
                                if c < 0:
                                    if with_bias:
                                        nc.tensor.matmul(
                                            psum[:],
                                            brow[0:1, col0:col0 + 128],
                                            ones[0:1, 0:512],
                                            start=False,
                                            stop=True,
                                        )
                                else:
                                    nc.tensor.matmul(
                                        psum[:],
                                        wqkv[:, c, col0:col0 + 128],
                                        xt[:, c, s * 512:(s + 1) * 512],
                                        start=(c == 0),
                                        stop=(not with_bias and c == DC - 1),
                                    )
                            if ci == 2:
                                nc.vector.tensor_copy(
                                    qt[:, p, s * 512:(s + 1) * 512], psum[:]
                                )
                        tasks.append(t)
                return tasks

            # ---- phases 2-4: attention groups. Scores drain to an SBUF
            # stage (DVE adds the mask bias there), exp runs as one big ACT op
            # per 4-kchunk batch, AV lags one batch, and background outproj /
            # Q-proj / V-proj tasks are woven between batches.
            for s in range(QC):
                for p in range(2):
                    # background tasks for this group, dispatched per kchunk
                    if s == 0 and p == 0:
                        tasks = {k: (lambda k=k: vproj_task(k)) for k in range(KC)}
                    elif s == 0 and p == 1:
                        qp = qproj_tasks(1)
                        tasks = {2 * ti + 2: t for ti, t in enumerate(qp)}
                    elif p == 0:
                        ot = outproj_tasks(s - 1)
                        tasks = {2 * ti + 1: t for ti, t in enumerate(ot)}
                    else:
                        qp = qproj_tasks(s + 1) if s + 1 < QC else []
                        tasks = {2 * ti + 2: t for ti, t in enumerate(qp)}
                    av = [
                        ps.tile([65, 512], f32, tag="ps", name=f"av{i}")
                        for i in range(2)
                    ]
                    exs = [None] * KC
                    for k in range(KC):
                        sc = ps_s.tile([128, 1024], f32, tag="spair", name="sc")
                        for i in range(2):
                            nc.tensor.matmul(
                                sc[:, i * 512:(i + 1) * 512],
                                kt[64 * i:64 * i + 64, p, k * 128:(k + 1) * 128],
                                qt[64 * i:64 * i + 64, p, s * 512:(s + 1) * 512],
                                start=True,
                                stop=True,
                            )
                        ex = exp_pool.tile([128, 1024], f16, tag="exp", name="ex")
                        nc.scalar.activation(ex[:], sc[:], AF.Exp, bias=mb[:, k:k + 1], scale=1.0)
                        exs[k] = ex
                        if k > 0:
                            for i in range(2):   # AV for iteration k-1 (pipelined)
                                nc.tensor.matmul(
                                    av[i][:],
                                    vn[:, k - 1, 2 * p + i, :],
                                    exs[k - 1][:, i * 512:(i + 1) * 512],
                                    start=(k - 1 == 0),
                                    stop=False,
                                )
                        if k in tasks:
                            tasks[k]()
                    for i in range(2):           # final AV (iteration KC-1)
                        nc.tensor.matmul(
                            av[i][:],
                            vn[:, KC - 1, 2 * p + i, :],
                            exs[KC - 1][:, i * 512:(i + 1) * 512],
                            start=False,
                            stop=True,
                        )
                    # normalize off the PE. Stash the denominator row and the
                    # unnormalized outT to SBUF first so the av PSUM slots free
                    # immediately; then approx-reciprocal + GpSimd partition
                    # broadcast + DVE multiply from the stashes.
                    stash = []
                    for i in range(2):
                        dnr = small_pool.tile([1, 512], f32, tag="dnr", name="dnr")
                        nc.vector.tensor_copy(dnr[:], av[i][64:65, :])
                        un = small_pool.tile([64, 512], f32, tag="un", name="un")
                        nc.vector.tensor_copy(un[:], av[i][0:64, :])
                        stash.append((dnr, un))
                    for i in range(2):
                        dnr, un = stash[i]
                        rcf = small_pool.tile([1, 512], f32, tag="rcf", name="rcf")
                        nc.vector.reciprocal_approx_fast(rcf[:], dnr[:])
                        if debug_taps:
                            r = 2 * (2 * s + p) + i
                            nc.sync.dma_start(dbg_dn[r:r + 1, :], dnr[:])
                            nc.sync.dma_start(dbg_rc[r:r + 1, :], rcf[:])
                        bc = small_pool.tile([64, 512], f32, tag="bc", name="bc")
                        nc.gpsimd.partition_broadcast(bc[:], rcf[:])
                        nc.vector.tensor_mul(
                            outt[64 * i:64 * i + 64, p, s * 512:(s + 1) * 512],
                            un[:],
                            bc[:],
                        )
            for t in outproj_tasks(QC - 1):
                t()

    nc.compile()
    _CACHE[key] = nc
    return nc


def make_in_maps(x, mask, W_qkv, b_qkv, W_out):
    x = np.asarray(x, np.float32)
    W_qkv = np.asarray(W_qkv, np.float32)
    b_qkv = np.asarray(b_qkv, np.float32)
    W_out = np.asarray(W_out, np.float32)
    if mask is None:
        m = np.ones((B, N), bool)
    else:
        mask = np.asarray(mask, bool)
        m = np.concatenate([np.ones((B, 1), bool), mask], axis=1)
    mb = np.where(m, np.float32(0.0), np.float32(-1e30)).astype(np.float32)

    in_maps = []
    for c in range(NCORES):
        b, g = divmod(c, GROUPS)
        cs = slice(DLOC * g, DLOC * g + DLOC)
        wq = W_qkv[:, 0:D][:, cs] * SCALE
        wk = W_qkv[:, D:2 * D][:, cs]
        wv = W_qkv[:, 2 * D:3 * D][:, cs]
        bq = b_qkv[0:D][cs] * SCALE
        bk = b_qkv[D:2 * D][cs]
        bv = np.zeros(DLOC, np.float32)   # V bias applied in combine()
        in_maps.append({
            "xt": np.ascontiguousarray(x[b].T).astype(np.float16),
            "wqkv": np.concatenate([wq, wk, wv], axis=1).astype(np.float16),
            "brow": np.concatenate([bq, bk, bv])[None, :].astype(np.float16),
            "wout": np.ascontiguousarray(W_out[cs, :]).astype(np.float16),
            "mb": np.ascontiguousarray(mb[b][:, None]),
            "ones_in": np.ones((128, 128), np.float16),
        })
    return in_maps


def combine(results, b_qkv, W_out, b_out):
    out = np.zeros((B, N, D), np.float32)
    for c in range(NCORES):
        out[c // GROUPS] += results[c]["y"]
    b_qkv = np.asarray(b_qkv, np.float32)
    W_out = np.asarray(W_out, np.float32)
    # attention rows sum to 1 -> V bias contributes b_v @ W_out everywhere
    out += (b_qkv[2 * D:3 * D] @ W_out)[None, None, :]
    out += np.asarray(b_out, np.float32)[None, None, :]
    return out


def kernel(x, mask=None, W_qkv=None, b_qkv=None, W_out=None, b_out=None, **kw):
    from concourse.bass_utils import run_bass_kernel_spmd

    qk_bias = np.any(np.asarray(b_qkv, np.float32)[0:2 * D])
    nc = build_model(with_bias=bool(qk_bias))
    in_maps = make_in_maps(x, mask, W_qkv, b_qkv, W_out)
    res = run_bass_kernel_spmd(nc, in_maps, core_ids=list(range(NCORES)))
    return combine(res.results, b_qkv, W_out, b_out)
